# revision 1
# baseline (speedup 1.0000x reference)
"""Trainium2 Bass kernel for a 6-layer transformer encoder.

Problem: B=4, S=512, D=1024, H=16 heads (depth 64), F=4096, L=6 layers, fp32.

Sharding: sequence-sharding within core pairs. Core c handles batch b=c//2,
token half r=c%2 (tokens r*256:(r+1)*256). Zero redundant FLOPs: each core
computes Q/K/V/attention/Wo/FFN/LN for its own 256 tokens only. Attention
needs all 512 keys, so K and V (computed per-half) are exchanged with ONE
combined pairwise AllGather per layer (~1MB bf16), issued right after the
K/V projections and hidden behind the Q projection.

Precision: weights are pre-cast to bf16 host-side (halves HBM traffic);
matmul operands are bf16, the residual stream (h, r1, h1, r2) stays f32r.
Softmax uses exp without max-subtraction (logits are O(1)), row sums via a
ones-column appended to V, and the mask folded into the exp bias.

Scheduling notes:
- PSUM: matmul outputs paired two-per-bank ([*,512] tiles); never mix
  operand base partitions within one bank, never interleave two open
  accumulation groups in one bank (both fault/corrupt on HW).
- LN stats matmuls are interleaved into the loops that produce r1/r2, so
  only the tiny scalar chain sits between phases.
- Attention-out normalize (recip/broadcast/scale) is pipelined inside the
  head loop with a lag of one tile to keep the PE queue warm.
"""

import numpy as np

TO = 256        # own tokens per core
S = 512         # total keys per batch element
D = 1024        # model dim
KD = D // 128   # 8 d-tiles
H = 16          # heads
DH = 64         # head dim
F = 4096        # ff dim
FT = F // 128   # 32 f-tiles
FH = FT // 2    # f-tiles per FFN half
L = 6           # layers
EPS = 1e-6
MAX_POS = 1000
NCORES = 8
KV_K = KD * TO              # 2048 bf16 cols of K payload
KV_V = 2 * H * 65           # 2080 bf16 cols of V payload
WBF = True      # bf16 weights + bf16 matmul operands; False = all-f32r

_cache = {}


def _imports():
    import sys
    try:
        import concourse.bass  # noqa
    except ImportError:
        for p in ("/opt/trn_rl_repo", "/root/.axon_site/_ro/trn_rl_repo"):
            if p not in sys.path:
                sys.path.insert(0, p)
    import concourse.bass as bass
    import concourse.mybir as mybir
    import concourse.tile as tile
    from concourse import bacc
    from concourse.bass_utils import run_bass_kernel_spmd
    return bass, mybir, tile, bacc, run_bass_kernel_spmd


def build(nlayers=L, use_cc=True, debug=False):
    bass, mybir, tile, bacc, _ = _imports()
    f32 = mybir.dt.float32
    f32r = mybir.dt.float32r
    bf16 = mybir.dt.bfloat16
    AF = mybir.ActivationFunctionType
    OP = mybir.AluOpType
    RG = [[0, 1], [2, 3], [4, 5], [6, 7]]

    nc = bacc.Bacc(None, target_bir_lowering=False, debug=True, num_devices=8)

    # ---- kernel I/O ----
    wdt = bf16 if WBF else f32r
    xT = nc.declare_dram_parameter("xT", [D, TO], f32r, isOutput=False)
    msk = nc.declare_dram_parameter("msk", [128, 4], f32, isOutput=False)
    # lhsT-packed: [l, t, kp, k, col] = W[l, k*128+kp, t*128+col]
    WqT = nc.declare_dram_parameter("WqT", [L, KD // 2, 128, 2, KD, 128], wdt,
                                    isOutput=False)
    WkT = nc.declare_dram_parameter("WkT", [L, KD // 2, 128, 2, KD, 128], wdt,
                                    isOutput=False)
    WoT = nc.declare_dram_parameter("WoT", [L, KD // 2, 128, 2, KD, 128], wdt,
                                    isOutput=False)
    # rhs-packed V: [l, nq, kp, k, col] = Wv[l, k*128+kp, nq*256+col]
    WvN = nc.declare_dram_parameter("WvN", [L, 4, 128, KD, 256], wdt, isOutput=False)
    # [l, fg, kp, k, col] = W1[l, k*128+kp, fg*128+col]
    W1T = nc.declare_dram_parameter("W1T", [L, FT // 4, 128, 4, KD, 128], wdt,
                                    isOutput=False)
    # [l, m, fp, fo, col] = W2[l, fo*128+fp, m*128+col]
    W2T = nc.declare_dram_parameter("W2T", [L, KD // 2, 128, 2, FT, 128], wdt,
                                    isOutput=False)
    bias9 = nc.declare_dram_parameter("bias9", [L, 128, KD, 9], f32, isOutput=False)
    b1h = nc.declare_dram_parameter("b1h", [L, 128, FT, 1], f32, isOutput=False)
    cst = nc.declare_dram_parameter("cst", [128, 65], f32r, isOutput=False)   # ones
    cstb = nc.declare_dram_parameter("cstb", [128, 32], bf16, isOutput=False)  # ones
    crow = nc.declare_dram_parameter("crow", [65, 128], f32r, isOutput=False)  # ones
    selc = nc.declare_dram_parameter("selc", [16, KD * 128], f32r, isOutput=False)
    out = nc.declare_dram_parameter("out", [D, TO], f32, isOutput=True)
    dbg = {}
    if debug:
        for name, shape in [("d_oT", [128, KD, TO]), ("d_h1", [128, KD, TO]),
                            ("d_u", [128, FH, TO]), ("d_r2", [128, KD, TO]),
                            ("d_sums", [16, TO]), ("d_r1", [128, KD, TO])]:
            dbg[name] = nc.declare_dram_parameter(name, shape, f32, isOutput=True)

    with tile.TileContext(nc) as tc:
        with tc.tile_pool(name="sb", bufs=1) as sb1, \
             tc.tile_pool(name="sb2", bufs=2) as sb2, \
             tc.tile_pool(name="sb3", bufs=3) as sb3, \
             tc.tile_pool(name="dram", bufs=2, space="DRAM") as dram, \
             tc.tile_pool(name="psA", bufs=3, space="PSUM") as psA, \
             tc.tile_pool(name="psB", bufs=2, space="PSUM") as psB:

            # ---- persistent tiles ----
            h = sb1.tile([128, KD, TO], f32r, tag="h")
            h_bf = sb1.tile([128, KD, TO], bf16, tag="h_bf", name="h_bf") if WBF else h
            cst_sb = sb1.tile([128, 65], f32r, tag="cst")
            crow_sb = sb1.tile([65, 128], f32r, tag="crow")
            msk_sb = sb1.tile([128, 4], f32, tag="msk")
            selc_sb = sb1.tile([16, KD * 128], f32r, tag="selc")
            # K for attention: [depth 64, head-in-pair 2, d-tile 8, key 512]
            # so every QK matmul reads base-partition 0.
            kTf = sb1.tile([64, 2, KD, S], bf16, tag="kTf")
            qTa64 = sb1.tile([64, 2, KD, TO], bf16, tag="qTa64")
            v1 = sb1.tile([128, 4, H, 65], bf16, tag="v1")          # full keys
            kT_own = sb1.tile([128, KD, TO], bf16, tag="kTown")
            v_own = sb1.tile([128, 2, H, 65], bf16, tag="vown")
            oT = sb1.tile([128, KD, TO], f32r, tag="oT")
            oTb = sb1.tile([128, KD, TO], bf16, tag="oTb", name="oTb") if WBF else oT
            h1 = sb1.tile([128, KD, TO], f32r, tag="h1")
            h1_bf = (sb1.tile([128, KD, TO], bf16, tag="h1_bf", name="h1_bf")
                     if WBF else h1)
            y2acc = sb1.tile([128, KD, TO], f32, tag="y2acc")
            r1 = sb1.tile([128, KD, TO], f32r, tag="r1")
            u = sb1.tile([128, FH, TO], bf16 if WBF else f32r, tag="u")
            qTa = sb1.tile([128, KD, TO], bf16, tag="qTa")
            sums16 = sb1.tile([16, TO], f32, tag="sums16")
            recIP = sb1.tile([16, TO], f32r, tag="recIP")

            nc.sync.dma_start(h[:], xT.rearrange("(ko kp) t -> kp ko t", kp=128))
            nc.sync.dma_start(cst_sb[:], cst[:])
            nc.sync.dma_start(crow_sb[:], crow[:])
            nc.sync.dma_start(msk_sb[:], msk[:])
            nc.sync.dma_start(selc_sb[:], selc[:])
            # ones column of v_own (written once; data writes never touch col 64)
            with nc.allow_non_contiguous_dma(reason="tiny one-time ones-column fill"):
                nc.sync.dma_start(v_own[:, :, :, 64], cstb[:])
            nc.gpsimd.memset(sums16[:], 1.0)
            # warm up the collective path (ENCD staging) with a tiny AllGather
            wrm_in = dram.tile([128, 16], bf16, tag="wrmin")
            nc.gpsimd.dma_start(wrm_in[:], cstb[:, 0:16])
            wrm_out = dram.tile([2, 128, 16], bf16, tag="wrmout")
            nc.gpsimd.collective_compute(
                "AllGather", OP.bypass, replica_groups=RG,
                ins=[wrm_in.opt()], outs=[wrm_out.opt()])
            if WBF:
                nc.scalar.activation(h_bf[:], h[:], AF.Copy)

            ones_col = cst_sb[:, 64:65]          # [128,1] f32r, stats lhsT
            onesr_ln = crow_sb[0:1, 0:128]       # [1,128] f32r @p0, LN bcast lhsT

            def proj_pair(wsrc, rhs_h, bias_fn, act_fn, tag="wsm", bufs=4):
                """Eight [128,TO] projections, paired two-per-PSUM-bank.

                Weight pairs load as one [128,2,KD,128] DMA on the sync queue.
                """
                for t2 in range(KD // 2):
                    wc = sb3.tile([128, 2, KD, 128], wdt, tag=tag, bufs=bufs)
                    nc.sync.dma_start(wc[:], wsrc(t2))
                    ps = psA.tile([128, 2 * TO], f32, tag="ps")
                    for half in range(2):
                        t = 2 * t2 + half
                        sl = ps[:, half * TO:(half + 1) * TO]
                        for k in range(KD):
                            nc.tensor.matmul(sl, wc[:, half, k, :], rhs_h[:, k, :],
                                             start=(k == 0), stop=(k == KD - 1))
                        act_fn(t, sl, bias_fn(t))

            def ln_begin():
                ps_s = psB.tile([1, TO], f32, tag="aux1", bufs=2, name="ps_s")
                ps_q = psB.tile([1, TO], f32, tag="aux1", bufs=2, name="ps_q")
                return ps_s, ps_q

            def ln_accum(st, o, rsl):
                """Accumulate sum / sum-of-squares of r's o-th tile (PE + DVE)."""
                ps_s, ps_q = st
                sq = sb2.tile([128, TO], f32r, tag="sq")
                with nc.allow_low_precision(reason="LN sq rounding"):
                    nc.vector.tensor_tensor(sq[:], rsl.bitcast(f32), rsl.bitcast(f32),
                                            OP.mult)
                nc.tensor.matmul(ps_s[:], ones_col, rsl, start=(o == 0),
                                 stop=(o == KD - 1))
                nc.tensor.matmul(ps_q[:], ones_col, sq[:], start=(o == 0),
                                 stop=(o == KD - 1))

            def ln_finish(st, r, dst, dst_bf, g_col, be_col):
                """dst = (r - mean) * rstd * g + be (f32r), dst_bf same in bf16."""
                ps_s, ps_q = st
                negm = sb2.tile([1, TO], f32r, tag="negm", bufs=1)
                with nc.allow_low_precision(reason="LN stats rounding"):
                    nc.vector.tensor_scalar(negm[:], ps_s[:], -1.0 / D, None, OP.mult)
                qs = sb2.tile([1, TO], f32, tag="lnscr", bufs=3)
                nc.vector.tensor_scalar(qs[:], ps_q[:], 1.0 / D, EPS, OP.mult, OP.add)
                msq = sb2.tile([1, TO], f32, tag="lnscr", bufs=3)
                nc.vector.tensor_tensor(msq[:], negm[:].bitcast(f32),
                                        negm[:].bitcast(f32), OP.mult)
                var = sb2.tile([1, TO], f32, tag="lnscr", bufs=3)
                nc.vector.tensor_tensor(var[:], qs[:], msq[:], OP.subtract)
                vrec = sb2.tile([1, TO], f32, tag="lnscr", bufs=3)
                nc.vector.reciprocal_approx_fast(vrec[:], var[:])
                rstd = sb2.tile([1, TO], f32r, tag="rstd", bufs=1)
                with nc.allow_low_precision(reason="LN rstd rounding"):
                    nc.scalar.activation(rstd[:], vrec[:], AF.Sqrt)
                bcast = psB.tile([128, 2 * TO], f32, tag="aux", bufs=1)
                pnm, prs = bcast[:, 0:TO], bcast[:, TO:2 * TO]
                nc.tensor.matmul(pnm, onesr_ln, negm[:], start=True, stop=True)
                nc.tensor.matmul(prs, onesr_ln, rstd[:], start=True, stop=True)
                for o in range(KD):
                    a = sb2.tile([128, TO], f32, tag="lna")
                    nc.vector.tensor_tensor(a[:], r[:, o, :].bitcast(f32), pnm, OP.add)
                    b = sb2.tile([128, TO], f32, tag="lnb")
                    nc.vector.tensor_tensor(b[:], a[:], prs, OP.mult)
                    with nc.allow_low_precision(reason="f32r LN output"):
                        nc.vector.tensor_scalar(dst[:, o, :], b[:], g_col[:, o, :],
                                                be_col[:, o, :], OP.mult, OP.add)
                    if dst_bf is not dst:
                        nc.scalar.activation(dst_bf[:, o, :], b[:], AF.Identity,
                                             bias=be_col[:, o, :],
                                             scale=g_col[:, o, :])

            def attn_norm(t, bia):
                """Per-head-pair softmax normalize + bv bias for d-tile t."""
                # full-16-row recip (DVE base partition must be 32-aligned);
                # the matmul below only reads the fresh rows 0:2t+2.
                with nc.allow_low_precision(reason="softmax recip rounding"):
                    nc.vector.reciprocal(recIP[:], sums16[:])
                prb = psB.tile([128, 2 * TO], f32, tag="aux", bufs=1, name="prb")
                # contract only over written recIP rows (selc rows >2t+1 are 0)
                nc.tensor.matmul(prb[:, 0:TO],
                                 selc_sb[0:2 * t + 2, t * 128:(t + 1) * 128],
                                 recIP[0:2 * t + 2, :], start=True, stop=True)
                with nc.allow_low_precision(reason="f32r attn normalize"):
                    nc.vector.tensor_tensor(oT[:, t, :], oT[:, t, :].bitcast(f32),
                                            prb[:, 0:TO], OP.mult)
                nc.scalar.activation(oTb[:, t, :], oT[:, t, :], AF.Identity,
                                     bias=bia[:, t, 2:3])

            for l in range(nlayers):
                # ---- per-layer bias/gain staging (host-packed) ----
                bia = sb2.tile([128, KD, 9], f32, tag="bias")
                nc.gpsimd.dma_start(bia[:], bias9[l])
                b1_sb = sb2.tile([128, FT, 1], f32, tag="b1")
                nc.gpsimd.dma_start(b1_sb[:], b1h[l])

                # ============ K + V projections (own tokens) ====================
                kv_in = dram.tile([128, KV_K + KV_V], bf16, tag="kvin")
                proj_pair(lambda t2: WkT[l, t2], h_bf,
                          lambda t: bia[:, t, 1:2],
                          lambda t, sl, b: nc.scalar.activation(
                              kT_own[:, t, :], sl, AF.Identity, bias=b))
                nc.gpsimd.dma_start(kv_in[:, 0:KV_K], kT_own[:])
                for nq in range(4):
                    wv_s = sb3.tile([128, KD, 256], wdt, tag="wvs", bufs=2)
                    nc.sync.dma_start(wv_s[:], WvN[l, nq])
                    pv = psA.tile([128, 2 * TO], f32, tag="ps")
                    for tt in range(2):
                        sl = pv[:, tt * 256:(tt + 1) * 256]
                        for k in range(KD):
                            nc.tensor.matmul(sl, h_bf[:, k, tt * 128:(tt + 1) * 128],
                                             wv_s[:, k, :], start=(k == 0),
                                             stop=(k == KD - 1))
                        nc.vector.tensor_copy(v_own[:, tt, nq * 4:(nq + 1) * 4, 0:64],
                                              sl)
                nc.gpsimd.dma_start(kv_in[:, KV_K:KV_K + KV_V], v_own[:])
                kv_out = dram.tile([2, 128, KV_K + KV_V], bf16, tag="kvout")
                if use_cc:
                    nc.gpsimd.collective_compute(
                        "AllGather", OP.bypass, replica_groups=RG,
                        ins=[kv_in.opt()], outs=[kv_out.opt()])
                else:
                    for r in range(2):
                        nc.sync.dma_start(kv_out[r], kv_in[:])

                # ================= Q projection (hides the AllGather) ===========
                def q_act(t, sl, b):
                    nc.scalar.activation(qTa[:, t, :], sl, AF.Identity, bias=b)
                    for hh in range(2):
                        nc.sync.dma_start(qTa64[:, hh, t, :],
                                          qTa[hh * 64:(hh + 1) * 64, t, :])
                proj_pair(lambda t2: WqT[l, t2], h_bf,
                          lambda t: bia[:, t, 0:1], q_act)

                # unpack gathered K/V (global key order: rank-major)
                for r in range(2):
                    for hh in range(2):
                        nc.sync.dma_start(kTf[:, hh, :, r * TO:(r + 1) * TO],
                                          kv_out[r, hh * 64:(hh + 1) * 64, 0:KV_K])
                    nc.sync.dma_start(v1[:, 2 * r:2 * r + 2, :, :],
                                      kv_out[r, :, KV_K:KV_K + KV_V])

                # ========================= attention ============================
                for t in range(KD):  # d-tile t covers heads 2t, 2t+1
                    po = psB.tile([65, 2 * TO], f32, tag="po")
                    eas = []
                    for kb in range(4):  # global key blocks (rank-major)
                        lt = psA.tile([128, 2 * TO], f32, tag="ps")
                        for hh in range(2):
                            nc.tensor.matmul(lt[:, hh * TO:(hh + 1) * TO],
                                             kTf[:, hh, t, kb * 128:(kb + 1) * 128],
                                             qTa64[:, hh, t, :], start=True, stop=True)
                        ea = sb2.tile([128, 2 * TO], bf16, tag="ea", bufs=4)
                        nc.scalar.activation(ea[:], lt[:], AF.Exp,
                                             bias=msk_sb[:, kb:kb + 1], scale=0.125)
                        eas.append(ea)
                    for pi in range(2):  # sequential accumulation groups per bank
                        for kb in range(4):
                            nc.tensor.matmul(po[:, pi * TO:(pi + 1) * TO],
                                             v1[:, kb, 2 * t + pi, :],
                                             eas[kb][:, pi * TO:(pi + 1) * TO],
                                             start=(kb == 0), stop=(kb == 3))
                        ov = sb2.tile([65, TO], f32, tag="ov")
                        nc.scalar.activation(ov[:], po[:, pi * TO:(pi + 1) * TO],
                                             AF.Copy)
                        nc.gpsimd.dma_start(
                            oT[pi * 64:pi * 64 + 64, t, :].bitcast(f32), ov[0:64, :])
                        nc.gpsimd.dma_start(sums16[2 * t + pi:2 * t + pi + 1, :],
                                            ov[64:65, :])
                    if t > 0:   # lag-1: normalize previous tile, PE queue stays warm
                        attn_norm(t - 1, bia)
                attn_norm(KD - 1, bia)

                # ============== Wo + residual + LN1 stats (interleaved) =========
                st1 = ln_begin()

                def wo_act(m, sl, b):
                    at = sb2.tile([128, TO], f32, tag="att")
                    nc.scalar.activation(at[:], sl, AF.Identity, bias=b)
                    with nc.allow_low_precision(reason="f32r residual"):
                        nc.vector.tensor_tensor(r1[:, m, :], at[:],
                                                h[:, m, :].bitcast(f32), OP.add)
                    ln_accum(st1, m, r1[:, m, :])
                proj_pair(lambda m2: WoT[l, m2], oTb,
                          lambda m: bia[:, m, 3:4], wo_act)
                if debug and l == 0:
                    nc.sync.dma_start(dbg["d_sums"][:], sums16[:])
                    nc.sync.dma_start(dbg["d_oT"][:], oT[:].bitcast(f32))
                    nc.sync.dma_start(dbg["d_r1"][:], r1[:].bitcast(f32))

                ln_finish(st1, r1, h1, h1_bf, bia[:, :, 5:6], bia[:, :, 6:7])

                # ============== FFN (two F-halves) + LN2 stats ==================
                st2 = ln_begin()
                for ph in range(2):
                    for fq in range(FH // 4):   # 4 f-groups per weight chunk
                        wc1 = sb3.tile([128, 4, KD, 128], wdt, tag="w1c", bufs=2)
                        nc.sync.dma_start(wc1[:], W1T[l, ph * (FH // 4) + fq])
                        for fg2 in range(2):
                            pu = psA.tile([128, 2 * TO], f32, tag="ps")
                            for half in range(2):
                                j = 2 * fg2 + half
                                fu = 4 * fq + j
                                fg = ph * FH + fu
                                sl = pu[:, half * TO:(half + 1) * TO]
                                for k in range(KD):
                                    nc.tensor.matmul(sl, wc1[:, j, k, :],
                                                     h1_bf[:, k, :],
                                                     start=(k == 0),
                                                     stop=(k == KD - 1))
                                nc.scalar.activation(u[:, fu, :], sl, AF.Relu,
                                                     bias=b1_sb[:, fg, 0:1])
                    for m2 in range(KD // 2):
                        wc2 = sb2.tile([128, 2, FH, 128], wdt, tag="w2c", bufs=2)
                        nc.sync.dma_start(
                            wc2[:], W2T[l, m2, :, :, ph * FH:(ph + 1) * FH, :])
                        py = psA.tile([128, 2 * TO], f32, tag="ps")
                        for half in range(2):
                            m = 2 * m2 + half
                            sl = py[:, half * TO:(half + 1) * TO]
                            for fo in range(FH):
                                nc.tensor.matmul(sl, wc2[:, half, fo, :], u[:, fo, :],
                                                 start=(fo == 0), stop=(fo == FH - 1))
                            if ph == 0:
                                nc.scalar.activation(y2acc[:, m, :].bitcast(f32r),
                                                     sl, AF.Identity,
                                                     bias=bia[:, m, 4:5])
                            else:
                                tmp = sb2.tile([128, TO], f32, tag="att")
                                nc.vector.tensor_tensor(tmp[:], sl,
                                                        h1[:, m, :].bitcast(f32),
                                                        OP.add)
                                with nc.allow_low_precision(reason="f32r residual"):
                                    nc.vector.tensor_tensor(
                                        y2acc[:, m, :].bitcast(f32r), tmp[:],
                                        y2acc[:, m, :], OP.add)
                                ln_accum(st2, m, y2acc[:, m, :].bitcast(f32r))
                r2 = y2acc[:].bitcast(f32r)
                if debug and l == 0:
                    nc.sync.dma_start(dbg["d_h1"][:], h1[:].bitcast(f32))
                    nc.sync.dma_start(dbg["d_u"][:], u[:].bitcast(f32))
                    nc.sync.dma_start(dbg["d_r2"][:], y2acc[:])

                ln_finish(st2, r2, h, h_bf, bia[:, :, 7:8], bia[:, :, 8:9])

            nc.sync.dma_start(out.rearrange("(ko kp) t -> kp ko t", kp=128),
                              h[:].bitcast(f32))

    nc.compile()
    return nc


def _selc():
    sel = np.zeros((16, KD * 128), np.float32)
    for t in range(KD):
        for m in range(128):
            sel[2 * t + m // 64, t * 128 + m] = 1.0
    return sel


def _pos_encoding(position, d):
    pos = np.arange(position)[:, None].astype(np.float32)
    i = np.arange(d)[None, :].astype(np.float32)
    angle = pos / np.power(10000.0, 2.0 * np.floor(i / 2.0) / np.float32(d))
    angle[:, 0::2] = np.sin(angle[:, 0::2])
    angle[:, 1::2] = np.cos(angle[:, 1::2])
    return angle.astype(np.float32)  # [position, d]


def _get_nc():
    if "nc" not in _cache:
        _cache["nc"] = build()
    return _cache["nc"]


def prepare_in_maps(inputs):
    """Host-side prep: pack weights/biases, shard tokens across 8 cores."""
    import ml_dtypes
    bf = ml_dtypes.bfloat16 if WBF else np.float32
    inp = {k: np.asarray(v, dtype=np.float32) for k, v in inputs.items()}
    pe = _pos_encoding(MAX_POS, D)[:S]
    x = inp["x"] + pe[None]

    def lhsT_pack(w):  # [L, D, D] -> [L, t2, kp, a, k, col]
        return np.ascontiguousarray(
            w.reshape(L, KD, 128, KD // 2, 2, 128).transpose(0, 3, 2, 4, 1, 5)
        ).astype(bf)

    common = {
        "WqT": lhsT_pack(inp["Wq"]),
        "WkT": lhsT_pack(inp["Wk"]),
        "WoT": lhsT_pack(inp["Wo"]),
        "WvN": np.ascontiguousarray(
            inp["Wv"].reshape(L, KD, 128, 4, 256).transpose(0, 3, 2, 1, 4)).astype(bf),
        "W1T": np.ascontiguousarray(
            inp["W1"].reshape(L, KD, 128, FT // 4, 4, 128)
            .transpose(0, 3, 2, 4, 1, 5)).astype(bf),
        "W2T": np.ascontiguousarray(
            inp["W2"].reshape(L, FT, 128, KD // 2, 2, 128)
            .transpose(0, 3, 2, 4, 1, 5)).astype(bf),
    }
    pk = lambda a: np.ascontiguousarray(a.reshape(L, KD, 128).transpose(0, 2, 1))
    common["bias9"] = np.ascontiguousarray(np.stack(
        [pk(inp[k]) for k in ["bq", "bk", "bv", "bo", "b2", "g1", "be1", "g2", "be2"]],
        axis=-1))
    common["b1h"] = np.ascontiguousarray(
        inp["b1"].reshape(L, FT, 128).transpose(0, 2, 1)[..., None])
    common["cst"] = np.ones((128, 65), np.float32)
    common["cstb"] = np.ones((128, 32), ml_dtypes.bfloat16)
    common["crow"] = np.ones((65, 128), np.float32)
    common["selc"] = _selc()
    in_maps = []
    for c in range(NCORES):
        b, r = c // 2, c % 2
        m = dict(common)
        m["xT"] = np.ascontiguousarray(x[b, r * TO:(r + 1) * TO, :].T)
        mk = (inp["mask"][b, 0, 0] * np.float32(-1e9)).astype(np.float32)
        m["msk"] = np.ascontiguousarray(mk.reshape(4, 128).T)
        in_maps.append(m)
    return in_maps


def kernel(**inputs):
    _, _, _, _, run_bass_kernel_spmd = _imports()
    nc = _get_nc()
    in_maps = prepare_in_maps(inputs)
    res = run_bass_kernel_spmd(nc, in_maps, core_ids=list(range(NCORES)))
    B = np.asarray(inputs["x"]).shape[0]
    out = np.stack([
        np.concatenate([res.results[2 * b]["out"].T,
                        res.results[2 * b + 1]["out"].T], axis=0)
        for b in range(B)])
    return out.astype(np.float32)

